# revision 36
# baseline (speedup 1.0000x reference)
"""Multi-head self-attention with RoPE on 8 Trainium2 NeuronCores.

Full inputs in, full output out. Sharding: batch (2) x head-groups (4 heads
per core). Each core computes qkv projections for its heads, RoPE, full
softmax(QK^T)V, and a combined (both head-pairs) partial output projection;
host sums the 4 partials per batch element and adds b_out.

All matmul operands are bf16 (fp32 PSUM accumulation); the emission order
interleaves the v projection and pair-1 q/k projections into pair-0's
ACT-bound attention stream so the PE stays busy.

Problem shape: B=2, T=2048, D=1024, H=16, HD=64 (hardcoded).
"""

import numpy as np
from contextlib import ExitStack

import ml_dtypes
import concourse.bass as bass
import concourse.mybir as mybir
import concourse.tile as tile
from concourse import bass_utils

B, T, D, H = 2, 2048, 1024, 16
HD = 64          # head dim
HL = 4           # heads per core
N_CORES = 8
ROPE_BASE = 10000.0

F32 = mybir.dt.float32
F32R = mybir.dt.float32r
BF16 = mybir.dt.bfloat16
BFNP = ml_dtypes.bfloat16

Exp = mybir.ActivationFunctionType.Exp

NT = T // 128     # 16 token tiles
NK = D // 128     # 8 contraction chunks
TH2 = 1024        # query-half width
SC = HD ** -0.5

# results of the last run (for test harness introspection)
LAST_RESULTS = None
TRACE = False


def _split_excess_waits(nc, cap=1):
    """walrus in this env rejects >1 sync-wait per instruction; split extras
    onto single-wait NoOps on the same engine queue."""
    n = 0
    for f in nc.m.functions:
        for bb in f.blocks:
            insts = bb.instructions
            if not any(
                i.sync_info is not None and len(i.sync_info.on_wait) > cap
                for i in insts
            ):
                continue
            out = []
            for inst in insts:
                si = inst.sync_info
                waits = list(si.on_wait) if si is not None else []
                if len(waits) > cap:
                    extra, keep = waits[:-cap], waits[-cap:]
                    for k, w in enumerate(extra):
                        nop = mybir.InstNoOp(
                            name=f"{inst.name}-ws{k}",
                            engine=inst.engine,
                            sync_info=mybir.SyncInfo(on_wait=[w], on_update=[]),
                            bass_nofuse=True,
                        )
                        nc.register_instruction(nop)
                        out.append(nop)
                        n += 1
                    inst.sync_info = mybir.SyncInfo(
                        on_wait=keep, on_update=list(si.on_update)
                    )
                out.append(inst)
            bb.instructions = out
    return n


def _build_bass(with_qkv_bias, with_v_bias):
    nc = bass.Bass("TRN2", target_bir_lowering=False, debug=False, num_devices=1)

    # ---- DRAM I/O ----
    d_xT = nc.dram_tensor("xT", [D, T], BF16, kind="ExternalInput").ap()
    d_wqk = nc.dram_tensor("wqk", [D, 4 * 128], BF16, kind="ExternalInput").ap()
    d_wv = nc.dram_tensor("wv", [D, HL * (HD + 1)], BF16, kind="ExternalInput").ap()
    d_bqk = nc.dram_tensor("bqk", [1, 4 * 128], BF16, kind="ExternalInput").ap()
    d_bv = nc.dram_tensor("bv", [1, HL * (HD + 1)], BF16, kind="ExternalInput").ap()
    d_ones = nc.dram_tensor("ones", [1, 512], BF16, kind="ExternalInput").ap()
    d_cos = nc.dram_tensor("cos2", [HD, T], F32, kind="ExternalInput").ap()
    d_sin = nc.dram_tensor("sin2", [HD, T], F32, kind="ExternalInput").ap()
    d_rT = nc.dram_tensor("rT", [128, 128], BF16, kind="ExternalInput").ap()
    d_ind = nc.dram_tensor("ind", [2, 128], F32R, kind="ExternalInput").ap()
    d_amask = nc.dram_tensor("amask", [128, NT], F32, kind="ExternalInput").ap()
    d_wo = nc.dram_tensor("wo", [2 * 128, D], BF16, kind="ExternalInput").ap()
    d_out = nc.dram_tensor("out_part", [T, D], BF16, kind="ExternalOutput").ap()

    with tile.TileContext(nc) as tc, ExitStack() as ctx:
        pool = lambda name, bufs: ctx.enter_context(tc.tile_pool(name=name, bufs=bufs))
        psum = lambda name, bufs: ctx.enter_context(
            tc.tile_pool(name=name, bufs=bufs, space="PSUM")
        )

        p_const = pool("const", 1)
        p_xt = pool("xt", NK)
        p_w = pool("w", NK)
        p_wv = pool("wv", NK)
        p_cs = pool("cs", 1)
        p_tmp = pool("tmp", 2)
        p_qk = pool("qk", 2)
        p_v = pool("v", NT)
        p_e = pool("e", 4)
        p_at = pool("at", 4)
        p_fin = pool("fin", 2)

        ps_s = psum("ps_s", 2)      # [128,1024] f32 -> 4 banks
        ps_pv = psum("ps_pv", 1)    # [65,1024] f32 -> 2 banks
        ps_aux = psum("ps_aux", 2)  # [128,512] f32 -> 2 banks

        # ---- input loads ----
        # x arrives in column-batches of 512 tokens via big rearranged
        # descriptors: batch qi unlocks the full contraction for token
        # quarter qi across every projection, so the PE starts ~8us in.
        # wqk rides the sync ring first; tables on the scalar ring.
        xt_all = p_xt.tile([128, NK * T], BF16, tag="xt", bufs=1, name="xt_all")
        wqk_all = p_w.tile([128, NK * 512], BF16, tag="wqk", bufs=1,
                           name="wqk_all")
        xt3 = xt_all[:].rearrange("p (c w) -> p c w", c=NK)
        xsrc = d_xT[:].rearrange("(c p) w -> p c w", p=128)
        # quarter 0 gates the whole prologue: cheap 2-D per-chunk
        # descriptors, wqk chunk interleaved with its x chunk so chunk-k
        # matmuls start as soon as pair k lands.
        for k in range(NK):
            nc.sync.dma_start(wqk_all[:, k * 512:(k + 1) * 512],
                              d_wqk[k * 128:(k + 1) * 128, :])
            nc.sync.dma_start(xt3[:, k, 0:512], xsrc[:, k, 0:512])
        for qi in range(1, 4):
            ws = slice(qi * 512, (qi + 1) * 512)
            nc.sync.dma_start(xt3[:, :, ws], xsrc[:, :, ws])

        def xt(k):
            return xt_all[:, k * T:(k + 1) * T]

        def wqk_sb(k):
            return wqk_all[:, k * 512:(k + 1) * 512]

        t_rT = p_const.tile([128, 128], BF16, tag="rT")
        nc.scalar.dma_start(t_rT[:], d_rT[:])
        t_cos = p_cs.tile([128, T], F32, tag="cos")
        t_sin = p_cs.tile([128, T], F32, tag="sin")
        nc.scalar.dma_start(t_sin[0:HD, :], d_sin[:])
        nc.scalar.dma_start(t_cos[0:HD, :], d_cos[:])
        nc.scalar.dma_start(t_sin[HD:128, :], t_sin[0:HD, :])
        nc.scalar.dma_start(t_cos[HD:128, :], t_cos[0:HD, :])
        t_amask = p_const.tile([128, NT], F32, tag="amask")
        nc.scalar.dma_start(t_amask[:], d_amask[:])
        t_indA = p_const.tile([1, 128], F32R, tag="indA")
        nc.scalar.dma_start(t_indA[:], d_ind[0:1, :])
        t_indB = p_const.tile([1, 128], F32R, tag="indB")
        nc.scalar.dma_start(t_indB[:], d_ind[1:2, :])
        t_ones = p_const.tile([1, 512], BF16, tag="ones")
        nc.scalar.dma_start(t_ones[:], d_ones[:])
        t_bqk = p_const.tile([1, 4 * 128], BF16, tag="bqk")
        nc.scalar.dma_start(t_bqk[:], d_bqk[:])
        t_bv = p_const.tile([1, HL * (HD + 1)], BF16, tag="bv")
        nc.scalar.dma_start(t_bv[:], d_bv[:])
        wv_all = p_wv.tile([128, NK * 260], BF16, tag="wv", bufs=1,
                           name="wv_all")
        nc.scalar.dma_start(
            wv_all[:].rearrange("p (c w) -> p c w", c=NK),
            d_wv[:].rearrange("(c p) w -> p c w", p=128),
        )

        def wv_sb(k):
            return wv_all[:, k * 260:(k + 1) * 260]

        # out-proj weights: not needed until late; sync ring after x
        wo_sb = []
        for c2 in range(2):
            wt = p_fin.tile([128, D], BF16, tag="wo", name="wo_t")
            nc.sync.dma_start(wt[:], d_wo[c2 * 128:(c2 + 1) * 128, :])
            wo_sb.append(wt)

        # ---- persistent q/k tiles; zero-pad k halves once ----
        qc, kA, kB = [], [], []
        for pair in range(2):
            tq = p_qk.tile([128, T], BF16, tag="qc", name="qc_t")
            ta = p_qk.tile([128, T], BF16, tag="kA", name="kA_t")
            tb = p_qk.tile([128, T], BF16, tag="kB", name="kB_t")
            nc.gpsimd.memset(ta[HD:128, :], 0.0)
            nc.gpsimd.memset(tb[0:HD, :], 0.0)
            qc.append(tq)
            kA.append(ta)
            kB.append(tb)

        v_sb = [None] * NT
        at_t = [None] * 4
        an_t = [None] * 4  # (pair, ih) -> 2*pair + ih

        # ---- emission helpers ----
        def emit_proj_mms(acc, c2, sl):
            for k in range(NK):
                nc.tensor.matmul(
                    acc,
                    wqk_sb(k)[:, c2 * 128:(c2 + 1) * 128],
                    xt(k)[:, sl],
                    start=(k == 0),
                    stop=(not with_qkv_bias and k == NK - 1),
                    skip_group_check=True,
                )
            if with_qkv_bias:
                nc.tensor.matmul(
                    acc,
                    t_bqk[:, c2 * 128:(c2 + 1) * 128],
                    t_ones[:, 0:512],
                    start=False,
                    stop=True,
                    skip_group_check=True,
                )

        def emit_rope(acc, qi, pair, is_k, rot_ring="aux"):
            """RoPE: roped = raw*cos + R @ (raw*sin); store q/k bf16."""
            sl = slice(qi * 512, (qi + 1) * 512)
            u = p_tmp.tile([128, 512], BF16, tag="u", name="u_t")
            nc.vector.tensor_mul(u[:], acc, t_sin[:, sl])
            if rot_ring == "pv":
                rot = ps_pv.tile([128, 512], F32, tag="pvA", name="rot")
            else:
                rot = ps_aux.tile([128, 512], F32, tag="aux", name="rot")
            nc.tensor.matmul(rot[:], t_rT[:], u[:], start=True, stop=True,
                             skip_group_check=True)
            c_sb = p_tmp.tile([128, 512], F32, tag="c", name="c_t")
            nc.vector.tensor_mul(c_sb[:], acc, t_cos[:, sl])
            if not is_k:
                nc.vector.tensor_add(qc[pair][:, sl], c_sb[:], rot[:])
            else:
                nc.vector.tensor_add(kA[pair][0:HD, sl], c_sb[0:HD, :],
                                     rot[0:HD, :])
                nc.vector.tensor_add(kB[pair][HD:128, sl], c_sb[HD:128, :],
                                     rot[HD:128, :])

        def emit_proj_quarter(c2, qi, pair, is_k):
            """interleaved-unit variant: acc+rot from the aux ring."""
            acc = ps_aux.tile([128, 512], F32, tag="aux", name="acc")
            emit_proj_mms(acc[:], c2, slice(qi * 512, (qi + 1) * 512))
            emit_rope(acc[:], qi, pair, is_k, rot_ring="aux")

        def emit_v_acc(j):
            acc = ps_aux.tile([128, 512], F32, tag="aux", name="vacc")
            av = acc[:, 0:HL * (HD + 1)]
            for k in range(NK):
                nc.tensor.matmul(
                    av,
                    xt(k)[:, j * 128:(j + 1) * 128],
                    wv_sb(k)[:],
                    start=(k == 0),
                    stop=(not with_v_bias and k == NK - 1),
                    skip_group_check=True,
                )
            if with_v_bias:
                nc.tensor.matmul(av, t_ones[:, 0:128], t_bv[:],
                                 start=False, stop=True, skip_group_check=True)
            return acc

        def emit_v_fin(j, acc):
            av = acc[:, 0:HL * (HD + 1)]
            vt = p_v.tile([128, HL * (HD + 1)], BF16, tag="v", name="v_t")
            nc.vector.tensor_copy(vt[:], av)
            if not with_v_bias:
                ones_cols = vt[:].rearrange("p (h c) -> p h c", h=HL)[:, :, HD:HD + 1]
                nc.gpsimd.memset(ones_cols, 1.0)
            v_sb[j] = vt

        def emit_v(j):
            emit_v_fin(j, emit_v_acc(j))

        norm_state = {}

        def emit_norm_head(pair, ih, hh):
            """per-head normalization prep right after head hh's attention
            half: reciprocal of the denominator row straight from PSUM, and
            (head B only) a partition-shift gather of its attention output."""
            hsl = slice(ih * TH2, (ih + 1) * TH2)
            at_ = at_t[2 * pair + hh]
            # reciprocal needs a partition-spread layout: gather the
            # denominator row to [128,8], recip, scatter to a flat row.
            sums = p_fin.tile([128, 8], F32, tag="sums", bufs=4, name="sums_t")
            nc.sync.dma_start(
                sums[:], at_[HD:HD + 1, hsl].rearrange("o (p c) -> o p c", p=128))
            rec = p_fin.tile([128, 8], F32, tag="rec", bufs=4, name="rec_t")
            nc.vector.reciprocal(rec[:], sums[:])
            rrow = p_fin.tile([1, TH2], F32R, tag="rrow", bufs=4, name="rrow_t")
            nc.sync.dma_start(
                rrow[:].rearrange("o (p c) -> o p c", p=128),
                rec[:].bitcast(F32R))
            if hh == 0:
                norm_state[(pair, ih)] = [rrow, None]
            else:
                norm_state[(pair, ih)][1] = rrow
                ar = p_fin.tile([128, TH2], F32, tag="ar", name="ar_t")
                nc.sync.dma_start(ar[HD:128, :], at_[0:HD, hsl])
                norm_state[(pair, ih)].append(ar)

        def emit_norm_fin(pair, ih):
            rrowA, rrowB, arB = norm_state.pop((pair, ih))
            at0 = at_t[2 * pair]
            hsl = slice(ih * TH2, (ih + 1) * TH2)
            an = p_fin.tile([128, TH2], BF16, tag="an", bufs=4, name="an_t")
            for n5 in range(2):
                s5 = slice(n5 * 512, (n5 + 1) * 512)
                g5 = slice(ih * TH2 + n5 * 512, ih * TH2 + (n5 + 1) * 512)
                pb = ps_aux.tile([128, 512], F32, tag="aux", name="pb")
                nc.tensor.matmul(pb[:], t_indA[:], rrowA[:, s5],
                                 start=True, stop=False, skip_group_check=True)
                nc.tensor.matmul(pb[:], t_indB[:], rrowB[:, s5],
                                 start=False, stop=True, skip_group_check=True)
                nc.vector.tensor_mul(an[0:HD, s5], pb[0:HD, :], at0[0:HD, g5])
                nc.vector.tensor_mul(an[HD:128, s5], pb[HD:128, :],
                                     arB[HD:128, s5])
            an_t[2 * pair + ih] = an

        def emit_outproj_tile(t, tail=False):
            """output projection for token tile t, both pairs accumulated."""
            ih = t // 8
            off = (t % 8) * 128
            an0, an1 = an_t[0 + ih], an_t[2 + ih]
            osb = p_fin.tile([128, D], BF16, tag="osb", bufs=4, name="osb_t")
            for n5 in range(2):
                s5 = slice(n5 * 512, (n5 + 1) * 512)
                pp = ps_aux.tile([128, 512], F32, tag="aux", name="pp")
                nc.tensor.matmul(pp[:], an0[:, off:off + 128], wo_sb[0][:, s5],
                                 start=True, stop=False, skip_group_check=True)
                nc.tensor.matmul(pp[:], an1[:, off:off + 128], wo_sb[1][:, s5],
                                 start=False, stop=True, skip_group_check=True)
                if tail and n5 == 1:
                    # ACT and DVE are both idle in the tail: split copies
                    nc.scalar.copy(osb[:, s5], pp[:])
                else:
                    nc.vector.tensor_copy(osb[:, s5], pp[:])
            nc.sync.dma_start(d_out[t * 128:(t + 1) * 128, :], osb[:])

        def emit_att_step(pair, ih, hh, jb):
            kp = (kA, kB)[hh][pair]
            s_ps = ps_s.tile([128, TH2], F32, tag="sT", name="s_ps")
            for n5 in range(2):
                s5 = slice(n5 * 512, (n5 + 1) * 512)
                g5 = slice(ih * TH2 + n5 * 512, ih * TH2 + (n5 + 1) * 512)
                nc.tensor.matmul(
                    s_ps[:, s5], kp[:, jb * 128:(jb + 1) * 128], qc[pair][:, g5],
                    start=True, stop=True, skip_group_check=True,
                )
            e = p_e.tile([128, TH2], BF16, tag="e", name="e_t")
            nc.scalar.activation(e[:], s_ps[:], Exp,
                                 bias=t_amask[:, jb:jb + 1], scale=SC)
            return s_ps, e

        def emit_pv(pair, hh, jb, pvA, pvB, e):
            h = 2 * pair + hh
            for n5, pvh in ((0, pvA), (1, pvB)):
                s5 = slice(n5 * 512, (n5 + 1) * 512)
                nc.tensor.matmul(
                    pvh[:],
                    v_sb[jb][:, h * (HD + 1):(h + 1) * (HD + 1)],
                    e[:, s5],
                    start=(jb == 0), stop=(jb == NT - 1),
                    skip_group_check=True,
                )

        # ---- prologue: k0 + q0 for token half 0 (x batches 0,1) ----
        # acc slots borrowed from the (still idle) scores ring so four
        # accumulations pipeline; rot slots borrowed from the pv ring.
        accs = []
        for qi in range(2):
            big = ps_s.tile([128, TH2], F32, tag="sT", name="acc_big")
            aK = big[:, 0:512]
            aQ = big[:, 512:1024]
            accs.append((aK, aQ))
            sl = slice(qi * 512, (qi + 1) * 512)
            for k in range(NK):
                last = not with_qkv_bias and k == NK - 1
                nc.tensor.matmul(aK, wqk_sb(k)[:, 2 * 128:3 * 128],
                                 xt(k)[:, sl], start=(k == 0), stop=last,
                                 skip_group_check=True)
                nc.tensor.matmul(aQ, wqk_sb(k)[:, 0:128],
                                 xt(k)[:, sl], start=(k == 0), stop=last,
                                 skip_group_check=True)
            if with_qkv_bias:
                nc.tensor.matmul(aK, t_bqk[:, 2 * 128:3 * 128],
                                 t_ones[:, 0:512], start=False, stop=True,
                                 skip_group_check=True)
                nc.tensor.matmul(aQ, t_bqk[:, 0:128],
                                 t_ones[:, 0:512], start=False, stop=True,
                                 skip_group_check=True)
        # v0/v1 accumulations next: they fill the PE while the DVE runs
        # batch-0's rope chains; then ropes, then the v copies (so the
        # copies queue on DVE *behind* the attention-gating chains).
        va0 = emit_v_acc(0)
        va1 = emit_v_acc(1)
        for qi in range(2):
            aK, aQ = accs[qi]
            emit_rope(aK, qi, 0, True, rot_ring="pv")
            emit_rope(aQ, qi, 0, False, rot_ring="pv")
        emit_v_fin(0, va0)
        emit_v_fin(1, va1)

        # pending interleave units for pair0's attention stream
        pend = []
        pend.append(lambda: emit_proj_quarter(2, 2, 0, True))   # k0 q2 (jb>=8)
        pend.append(lambda: emit_proj_quarter(2, 3, 0, True))   # k0 q3 (jb>=12)
        pend.append(lambda: emit_proj_quarter(0, 2, 0, False))  # q0 ih1
        pend.append(lambda: emit_proj_quarter(0, 3, 0, False))
        for qi in range(4):
            pend.append(lambda qi=qi: emit_proj_quarter(3, qi, 1, True))   # k1
        for qi in range(4):
            pend.append(lambda qi=qi: emit_proj_quarter(1, qi, 1, False))  # q1

        def drain(n=1):
            for _ in range(n):
                if pend:
                    pend.pop(0)()

        # deferred norm-fins: the ind-matmuls wait ~3us on the reciprocal
        # DMA round-trip; firing them 2 steps into the NEXT quarter keeps
        # them out of the PE queue's critical path.
        fin_box = [None]

        def maybe_fin():
            if fin_box[0] is not None:
                emit_norm_fin(*fin_box[0])
                fin_box[0] = None

        def run_quarter(pair, ih, hh, extra):
            if ih == 0:
                at_t[2 * pair + hh] = p_at.tile([HD + 1, T], F32, tag="aT",
                                                name="at_t")
            at = at_t[2 * pair + hh]
            pvA = ps_pv.tile([HD + 1, 512], F32, tag="pvA", name="pvA_t")
            pvB = ps_pv.tile([HD + 1, 512], F32, tag="pvB", name="pvB_t")
            for jb in range(NT):
                s_ps, e = emit_att_step(pair, ih, hh, jb)
                if jb == 4:
                    maybe_fin()
                extra(jb)
                emit_pv(pair, hh, jb, pvA, pvB, e)
            h0 = ih * TH2
            nc.vector.tensor_copy(at[:, h0:h0 + 512], pvA[:])
            nc.vector.tensor_copy(at[:, h0 + 512:h0 + TH2], pvB[:])
            emit_norm_head(pair, ih, hh)

        # ---- pair 0 attention ----
        def p0_extra(ih, hh):
            def f(jb):
                if ih == 0 and hh == 0:
                    if 1 <= jb < NT - 1:
                        emit_v(jb + 1)
                    if jb in (1, 5, 9, 13):
                        drain(1)  # k0 q2/q3 ahead of jb 8/12, then q0 ih1
                elif jb % 4 == 0:
                    drain(1)  # k1/q1 quarters, evenly spread
            return f

        for ih in range(2):
            for hh in range(2):
                run_quarter(0, ih, hh, p0_extra(ih, hh))
            fin_box[0] = (0, ih)

        # ---- pair 1 attention ----
        OUTPROJ_SCHED = {(0, 6): 0, (0, 10): 1, (0, 14): 2,
                         (1, 0): 3, (1, 4): 4, (1, 8): 5, (1, 12): 6,
                         (1, 14): 7}

        def p1_extra(ih, hh):
            def f(jb):
                if ih == 0 and jb % 8 == 4:
                    drain(1)  # any leftover proj units
                if ih == 1 and (hh, jb) in OUTPROJ_SCHED:
                    emit_outproj_tile(OUTPROJ_SCHED[(hh, jb)])
            return f

        for ih in range(2):
            for hh in range(2):
                run_quarter(1, ih, hh, p1_extra(ih, hh))
            fin_box[0] = (1, ih)

        # ---- tail: last norm + remaining outproj ----
        maybe_fin()
        for t in range(8, NT):
            emit_outproj_tile(t, tail=True)

    _split_excess_waits(nc)
    return nc


_NC_CACHE = {}


def _rope_tables():
    inv_freq = (1.0 / (ROPE_BASE ** (np.arange(0, HD, 2, dtype=np.float32) / HD))
                ).astype(np.float32)
    t = np.arange(T, dtype=np.float32)
    freqs = np.einsum("t,f->tf", t, inv_freq).astype(np.float32)  # (T, HD/2)
    emb = np.concatenate([freqs, freqs], axis=-1)                  # (T, HD)
    cosT = np.ascontiguousarray(np.cos(emb).astype(np.float32).T)  # (HD, T)
    sinT = np.ascontiguousarray(np.sin(emb).astype(np.float32).T)
    return cosT, sinT


def _rot_matrix():
    r = np.zeros((128, 128), dtype=np.float32)
    for p0 in (0, 64):
        for d in range(32):
            r[p0 + d, p0 + 32 + d] = -1.0
            r[p0 + 32 + d, p0 + d] = 1.0
    return np.ascontiguousarray(r.T)


def kernel(x, W_qkv, b_qkv, W_out, b_out, padding_mask):
    global _NC_CACHE, LAST_RESULTS
    x = np.asarray(x, dtype=np.float32)
    W_qkv = np.asarray(W_qkv, dtype=np.float32)
    b_qkv = np.asarray(b_qkv, dtype=np.float32)
    W_out = np.asarray(W_out, dtype=np.float32)
    b_out = np.asarray(b_out, dtype=np.float32)
    padding_mask = np.asarray(padding_mask)

    with_qkv_bias = bool(np.any(b_qkv[:2 * D]))
    with_v_bias = bool(np.any(b_qkv[2 * D:]))
    key = (with_qkv_bias, with_v_bias)
    if key not in _NC_CACHE:
        _NC_CACHE[key] = _build_bass(with_qkv_bias, with_v_bias)
    nc = _NC_CACHE[key]

    cos2, sin2 = _rope_tables()
    rT = _rot_matrix().astype(BFNP)

    ind = np.zeros((2, 128), dtype=np.float32)
    for f in range(128):
        ind[f // 64, f] = 1.0

    ones = np.ones((1, 512), dtype=BFNP)

    in_maps = []
    for c in range(N_CORES):
        b = c // 4
        g = c % 4
        q0 = g * HL * HD
        wq = W_qkv[:, q0:q0 + HL * HD]
        wk = W_qkv[:, D + q0:D + q0 + HL * HD]
        wv_flat = W_qkv[:, 2 * D + q0:2 * D + q0 + HL * HD]
        # interleave v columns with a zero (ones-slot) column per head
        wv_aug = np.zeros((D, HL * (HD + 1)), dtype=np.float32)
        bv_aug = np.zeros((1, HL * (HD + 1)), dtype=np.float32)
        for h in range(HL):
            wv_aug[:, h * (HD + 1):h * (HD + 1) + HD] = wv_flat[:, h * HD:(h + 1) * HD]
            bv_aug[0, h * (HD + 1):h * (HD + 1) + HD] = \
                b_qkv[2 * D + q0 + h * HD:2 * D + q0 + (h + 1) * HD]
            bv_aug[0, h * (HD + 1) + HD] = 1.0
        bqk = np.concatenate(
            [b_qkv[q0:q0 + HL * HD], b_qkv[D + q0:D + q0 + HL * HD]]
        ).reshape(1, -1).astype(np.float32)
        amask = np.where(padding_mask[b], np.float32(-1e30), np.float32(0.0))
        amask = np.ascontiguousarray(amask.reshape(T // 128, 128).T.astype(np.float32))
        in_maps.append({
            "xT": np.ascontiguousarray(x[b].T).astype(BFNP),
            "wqk": np.ascontiguousarray(
                np.concatenate([wq, wk], axis=1)).astype(BFNP),
            "wv": wv_aug.astype(BFNP),
            "bqk": bqk.astype(BFNP),
            "bv": bv_aug.astype(BFNP),
            "ones": ones,
            "cos2": cos2,
            "sin2": sin2,
            "rT": rT,
            "ind": ind,
            "amask": amask,
            "wo": np.ascontiguousarray(W_out[q0:q0 + HL * HD, :]).astype(BFNP),
        })

    res = bass_utils.run_bass_kernel_spmd(
        nc, in_maps, core_ids=list(range(N_CORES)), trace=TRACE,
    )
    LAST_RESULTS = res

    out = np.zeros((B, T, D), dtype=np.float32)
    for c in range(N_CORES):
        out[c // 4] += res.results[c]["out_part"].astype(np.float32)
    out += b_out.astype(np.float32)
    return out.astype(np.float32)


# revision 37
# speedup vs baseline: 1.0125x; 1.0125x over previous
"""Multi-head self-attention with RoPE on 8 Trainium2 NeuronCores.

Full inputs in, full output out. Sharding: batch (2) x head-groups (4 heads
per core). Each core computes qkv projections for its heads, RoPE, full
softmax(QK^T)V, and a combined (both head-pairs) partial output projection;
host sums the 4 partials per batch element and adds b_out.

All matmul operands are bf16 (fp32 PSUM accumulation); the emission order
interleaves the v projection and pair-1 q/k projections into pair-0's
ACT-bound attention stream so the PE stays busy.

Problem shape: B=2, T=2048, D=1024, H=16, HD=64 (hardcoded).
"""

import numpy as np
from contextlib import ExitStack

import ml_dtypes
import concourse.bass as bass
import concourse.mybir as mybir
import concourse.tile as tile
from concourse import bass_utils

B, T, D, H = 2, 2048, 1024, 16
HD = 64          # head dim
HL = 4           # heads per core
N_CORES = 8
ROPE_BASE = 10000.0

F32 = mybir.dt.float32
F32R = mybir.dt.float32r
BF16 = mybir.dt.bfloat16
BFNP = ml_dtypes.bfloat16

Exp = mybir.ActivationFunctionType.Exp

NT = T // 128     # 16 token tiles
NK = D // 128     # 8 contraction chunks
TH2 = 1024        # query-half width
SC = HD ** -0.5

# results of the last run (for test harness introspection)
LAST_RESULTS = None
TRACE = False


def _split_excess_waits(nc, cap=1):
    """walrus in this env rejects >1 sync-wait per instruction; split extras
    onto single-wait NoOps on the same engine queue."""
    n = 0
    for f in nc.m.functions:
        for bb in f.blocks:
            insts = bb.instructions
            if not any(
                i.sync_info is not None and len(i.sync_info.on_wait) > cap
                for i in insts
            ):
                continue
            out = []
            for inst in insts:
                si = inst.sync_info
                waits = list(si.on_wait) if si is not None else []
                if len(waits) > cap:
                    extra, keep = waits[:-cap], waits[-cap:]
                    for k, w in enumerate(extra):
                        nop = mybir.InstNoOp(
                            name=f"{inst.name}-ws{k}",
                            engine=inst.engine,
                            sync_info=mybir.SyncInfo(on_wait=[w], on_update=[]),
                            bass_nofuse=True,
                        )
                        nc.register_instruction(nop)
                        out.append(nop)
                        n += 1
                    inst.sync_info = mybir.SyncInfo(
                        on_wait=keep, on_update=list(si.on_update)
                    )
                out.append(inst)
            bb.instructions = out
    return n


def _build_bass(with_qkv_bias, with_v_bias):
    nc = bass.Bass("TRN2", target_bir_lowering=False, debug=False, num_devices=1)

    # ---- DRAM I/O ----
    d_xT = nc.dram_tensor("xT", [D, T], BF16, kind="ExternalInput").ap()
    d_wqk = nc.dram_tensor("wqk", [D, 4 * 128], BF16, kind="ExternalInput").ap()
    d_wv = nc.dram_tensor("wv", [D, HL * (HD + 1)], BF16, kind="ExternalInput").ap()
    d_bqk = nc.dram_tensor("bqk", [1, 4 * 128], BF16, kind="ExternalInput").ap()
    d_bv = nc.dram_tensor("bv", [1, HL * (HD + 1)], BF16, kind="ExternalInput").ap()
    d_ones = nc.dram_tensor("ones", [1, 512], BF16, kind="ExternalInput").ap()
    d_cos = nc.dram_tensor("cos2", [HD, T], F32, kind="ExternalInput").ap()
    d_sin = nc.dram_tensor("sin2", [HD, T], F32, kind="ExternalInput").ap()
    d_rT = nc.dram_tensor("rT", [128, 128], BF16, kind="ExternalInput").ap()
    d_ind = nc.dram_tensor("ind", [2, 128], F32R, kind="ExternalInput").ap()
    d_amask = nc.dram_tensor("amask", [128, NT], F32, kind="ExternalInput").ap()
    d_wo = nc.dram_tensor("wo", [2 * 128, D], BF16, kind="ExternalInput").ap()
    d_out = nc.dram_tensor("out_part", [T, D], BF16, kind="ExternalOutput").ap()

    with tile.TileContext(nc) as tc, ExitStack() as ctx:
        pool = lambda name, bufs: ctx.enter_context(tc.tile_pool(name=name, bufs=bufs))
        psum = lambda name, bufs: ctx.enter_context(
            tc.tile_pool(name=name, bufs=bufs, space="PSUM")
        )

        p_const = pool("const", 1)
        p_xt = pool("xt", NK)
        p_w = pool("w", NK)
        p_wv = pool("wv", NK)
        p_cs = pool("cs", 1)
        p_tmp = pool("tmp", 2)
        p_qk = pool("qk", 2)
        p_v = pool("v", NT)
        p_e = pool("e", 4)
        p_at = pool("at", 4)
        p_fin = pool("fin", 2)

        ps_s = psum("ps_s", 2)      # [128,1024] f32 -> 4 banks
        ps_pv = psum("ps_pv", 1)    # [65,1024] f32 -> 2 banks
        ps_aux = psum("ps_aux", 2)  # [128,512] f32 -> 2 banks

        # ---- input loads ----
        # x arrives in column-batches of 512 tokens via big rearranged
        # descriptors: batch qi unlocks the full contraction for token
        # quarter qi across every projection, so the PE starts ~8us in.
        # wqk rides the sync ring first; tables on the scalar ring.
        xt_all = p_xt.tile([128, NK * T], BF16, tag="xt", bufs=1, name="xt_all")
        wqk_all = p_w.tile([128, NK * 512], BF16, tag="wqk", bufs=1,
                           name="wqk_all")
        xt3 = xt_all[:].rearrange("p (c w) -> p c w", c=NK)
        xsrc = d_xT[:].rearrange("(c p) w -> p c w", p=128)
        # quarter 0 gates the whole prologue: cheap 2-D per-chunk
        # descriptors, wqk chunk interleaved with its x chunk so chunk-k
        # matmuls start as soon as pair k lands.
        for k in range(NK):
            nc.sync.dma_start(wqk_all[:, k * 512:(k + 1) * 512],
                              d_wqk[k * 128:(k + 1) * 128, :])
            nc.sync.dma_start(xt3[:, k, 0:512], xsrc[:, k, 0:512])
        for qi in range(1, 4):
            ws = slice(qi * 512, (qi + 1) * 512)
            nc.sync.dma_start(xt3[:, :, ws], xsrc[:, :, ws])

        def xt(k):
            return xt_all[:, k * T:(k + 1) * T]

        def wqk_sb(k):
            return wqk_all[:, k * 512:(k + 1) * 512]

        t_rT = p_const.tile([128, 128], BF16, tag="rT")
        nc.scalar.dma_start(t_rT[:], d_rT[:])
        t_cos = p_cs.tile([128, T], F32, tag="cos")
        t_sin = p_cs.tile([128, T], F32, tag="sin")
        nc.scalar.dma_start(t_sin[0:HD, :], d_sin[:])
        nc.scalar.dma_start(t_cos[0:HD, :], d_cos[:])
        nc.scalar.dma_start(t_sin[HD:128, :], t_sin[0:HD, :])
        nc.scalar.dma_start(t_cos[HD:128, :], t_cos[0:HD, :])
        t_amask = p_const.tile([128, NT], F32, tag="amask")
        nc.scalar.dma_start(t_amask[:], d_amask[:])
        t_indA = p_const.tile([1, 128], F32R, tag="indA")
        nc.scalar.dma_start(t_indA[:], d_ind[0:1, :])
        t_indB = p_const.tile([1, 128], F32R, tag="indB")
        nc.scalar.dma_start(t_indB[:], d_ind[1:2, :])
        t_ones = p_const.tile([1, 512], BF16, tag="ones")
        nc.scalar.dma_start(t_ones[:], d_ones[:])
        t_bqk = p_const.tile([1, 4 * 128], BF16, tag="bqk")
        nc.scalar.dma_start(t_bqk[:], d_bqk[:])
        t_bv = p_const.tile([1, HL * (HD + 1)], BF16, tag="bv")
        nc.scalar.dma_start(t_bv[:], d_bv[:])
        wv_all = p_wv.tile([128, NK * 260], BF16, tag="wv", bufs=1,
                           name="wv_all")
        nc.scalar.dma_start(
            wv_all[:].rearrange("p (c w) -> p c w", c=NK),
            d_wv[:].rearrange("(c p) w -> p c w", p=128),
        )

        def wv_sb(k):
            return wv_all[:, k * 260:(k + 1) * 260]

        # out-proj weights: not needed until late; sync ring after x
        wo_sb = []
        for c2 in range(2):
            wt = p_fin.tile([128, D], BF16, tag="wo", name="wo_t")
            nc.sync.dma_start(wt[:], d_wo[c2 * 128:(c2 + 1) * 128, :])
            wo_sb.append(wt)

        # ---- persistent q/k tiles; zero-pad k halves once ----
        qc, kA, kB = [], [], []
        for pair in range(2):
            tq = p_qk.tile([128, T], BF16, tag="qc", name="qc_t")
            ta = p_qk.tile([128, T], BF16, tag="kA", name="kA_t")
            tb = p_qk.tile([128, T], BF16, tag="kB", name="kB_t")
            nc.gpsimd.memset(ta[HD:128, :], 0.0)
            nc.gpsimd.memset(tb[0:HD, :], 0.0)
            qc.append(tq)
            kA.append(ta)
            kB.append(tb)

        v_sb = [None] * NT
        at_t = [None] * 4
        an_t = [None] * 4  # (pair, ih) -> 2*pair + ih

        # ---- emission helpers ----
        def emit_proj_mms(acc, c2, sl):
            for k in range(NK):
                nc.tensor.matmul(
                    acc,
                    wqk_sb(k)[:, c2 * 128:(c2 + 1) * 128],
                    xt(k)[:, sl],
                    start=(k == 0),
                    stop=(not with_qkv_bias and k == NK - 1),
                    skip_group_check=True,
                )
            if with_qkv_bias:
                nc.tensor.matmul(
                    acc,
                    t_bqk[:, c2 * 128:(c2 + 1) * 128],
                    t_ones[:, 0:512],
                    start=False,
                    stop=True,
                    skip_group_check=True,
                )

        def emit_rope(acc, qi, pair, is_k, rot_ring="aux"):
            """RoPE: roped = raw*cos + R @ (raw*sin); store q/k bf16."""
            sl = slice(qi * 512, (qi + 1) * 512)
            u = p_tmp.tile([128, 512], BF16, tag="u", name="u_t")
            nc.vector.tensor_mul(u[:], acc, t_sin[:, sl])
            if rot_ring == "pv":
                rot = ps_pv.tile([128, 512], F32, tag="pvA", name="rot")
            else:
                rot = ps_aux.tile([128, 512], F32, tag="aux", name="rot")
            nc.tensor.matmul(rot[:], t_rT[:], u[:], start=True, stop=True,
                             skip_group_check=True)
            c_sb = p_tmp.tile([128, 512], F32, tag="c", name="c_t")
            nc.vector.tensor_mul(c_sb[:], acc, t_cos[:, sl])
            if not is_k:
                nc.vector.tensor_add(qc[pair][:, sl], c_sb[:], rot[:])
            else:
                nc.vector.tensor_add(kA[pair][0:HD, sl], c_sb[0:HD, :],
                                     rot[0:HD, :])
                nc.vector.tensor_add(kB[pair][HD:128, sl], c_sb[HD:128, :],
                                     rot[HD:128, :])

        def emit_proj_quarter(c2, qi, pair, is_k):
            """interleaved-unit variant: acc+rot from the aux ring."""
            acc = ps_aux.tile([128, 512], F32, tag="aux", name="acc")
            emit_proj_mms(acc[:], c2, slice(qi * 512, (qi + 1) * 512))
            emit_rope(acc[:], qi, pair, is_k, rot_ring="aux")

        def emit_v_acc(j):
            acc = ps_aux.tile([128, 512], F32, tag="aux", name="vacc")
            av = acc[:, 0:HL * (HD + 1)]
            for k in range(NK):
                nc.tensor.matmul(
                    av,
                    xt(k)[:, j * 128:(j + 1) * 128],
                    wv_sb(k)[:],
                    start=(k == 0),
                    stop=(not with_v_bias and k == NK - 1),
                    skip_group_check=True,
                )
            if with_v_bias:
                nc.tensor.matmul(av, t_ones[:, 0:128], t_bv[:],
                                 start=False, stop=True, skip_group_check=True)
            return acc

        def emit_v_fin(j, acc):
            av = acc[:, 0:HL * (HD + 1)]
            vt = p_v.tile([128, HL * (HD + 1)], BF16, tag="v", name="v_t")
            nc.vector.tensor_copy(vt[:], av)
            if not with_v_bias:
                ones_cols = vt[:].rearrange("p (h c) -> p h c", h=HL)[:, :, HD:HD + 1]
                nc.gpsimd.memset(ones_cols, 1.0)
            v_sb[j] = vt

        def emit_v(j):
            emit_v_fin(j, emit_v_acc(j))

        norm_state = {}

        def emit_norm_head(pair, ih, hh):
            """per-head normalization prep right after head hh's attention
            half: reciprocal of the denominator row straight from PSUM, and
            (head B only) a partition-shift gather of its attention output."""
            hsl = slice(ih * TH2, (ih + 1) * TH2)
            at_ = at_t[2 * pair + hh]
            # reciprocal needs a partition-spread layout: gather the
            # denominator row to [128,8], recip, scatter to a flat row.
            sums = p_fin.tile([128, 8], F32, tag="sums", bufs=4, name="sums_t")
            nc.sync.dma_start(
                sums[:], at_[HD:HD + 1, hsl].rearrange("o (p c) -> o p c", p=128))
            rec = p_fin.tile([128, 8], F32, tag="rec", bufs=4, name="rec_t")
            nc.vector.reciprocal(rec[:], sums[:])
            rrow = p_fin.tile([1, TH2], F32R, tag="rrow", bufs=4, name="rrow_t")
            nc.sync.dma_start(
                rrow[:].rearrange("o (p c) -> o p c", p=128),
                rec[:].bitcast(F32R))
            if hh == 0:
                norm_state[(pair, ih)] = [rrow, None]
            else:
                norm_state[(pair, ih)][1] = rrow
                ar = p_fin.tile([128, TH2], F32, tag="ar", name="ar_t")
                nc.sync.dma_start(ar[HD:128, :], at_[0:HD, hsl])
                norm_state[(pair, ih)].append(ar)

        def emit_norm_fin(pair, ih):
            rrowA, rrowB, arB = norm_state.pop((pair, ih))
            at0 = at_t[2 * pair]
            hsl = slice(ih * TH2, (ih + 1) * TH2)
            an = p_fin.tile([128, TH2], BF16, tag="an", bufs=4, name="an_t")
            for n5 in range(2):
                s5 = slice(n5 * 512, (n5 + 1) * 512)
                g5 = slice(ih * TH2 + n5 * 512, ih * TH2 + (n5 + 1) * 512)
                pb = ps_aux.tile([128, 512], F32, tag="aux", name="pb")
                nc.tensor.matmul(pb[:], t_indA[:], rrowA[:, s5],
                                 start=True, stop=False, skip_group_check=True)
                nc.tensor.matmul(pb[:], t_indB[:], rrowB[:, s5],
                                 start=False, stop=True, skip_group_check=True)
                nc.vector.tensor_mul(an[0:HD, s5], pb[0:HD, :], at0[0:HD, g5])
                nc.vector.tensor_mul(an[HD:128, s5], pb[HD:128, :],
                                     arB[HD:128, s5])
            an_t[2 * pair + ih] = an

        def emit_outproj_tile(t, tail=False):
            """output projection for token tile t, both pairs accumulated."""
            ih = t // 8
            off = (t % 8) * 128
            an0, an1 = an_t[0 + ih], an_t[2 + ih]
            osb = p_fin.tile([128, D], BF16, tag="osb", bufs=4, name="osb_t")
            for n5 in range(2):
                s5 = slice(n5 * 512, (n5 + 1) * 512)
                pp = ps_aux.tile([128, 512], F32, tag="aux", name="pp")
                nc.tensor.matmul(pp[:], an0[:, off:off + 128], wo_sb[0][:, s5],
                                 start=True, stop=False, skip_group_check=True)
                nc.tensor.matmul(pp[:], an1[:, off:off + 128], wo_sb[1][:, s5],
                                 start=False, stop=True, skip_group_check=True)
                if tail and n5 == 1:
                    # ACT and DVE are both idle in the tail: split copies
                    nc.scalar.copy(osb[:, s5], pp[:])
                else:
                    nc.vector.tensor_copy(osb[:, s5], pp[:])
            nc.sync.dma_start(d_out[t * 128:(t + 1) * 128, :], osb[:])

        def emit_att_step(pair, ih, hh, jb):
            kp = (kA, kB)[hh][pair]
            s_ps = ps_s.tile([128, TH2], F32, tag="sT", name="s_ps")
            for n5 in range(2):
                s5 = slice(n5 * 512, (n5 + 1) * 512)
                g5 = slice(ih * TH2 + n5 * 512, ih * TH2 + (n5 + 1) * 512)
                nc.tensor.matmul(
                    s_ps[:, s5], kp[:, jb * 128:(jb + 1) * 128], qc[pair][:, g5],
                    start=True, stop=True, skip_group_check=True,
                )
            e = p_e.tile([128, TH2], BF16, tag="e", name="e_t")
            nc.scalar.activation(e[:], s_ps[:], Exp,
                                 bias=t_amask[:, jb:jb + 1], scale=SC)
            return s_ps, e

        def emit_pv(pair, hh, jb, pvA, pvB, e):
            h = 2 * pair + hh
            for n5, pvh in ((0, pvA), (1, pvB)):
                s5 = slice(n5 * 512, (n5 + 1) * 512)
                nc.tensor.matmul(
                    pvh[:],
                    v_sb[jb][:, h * (HD + 1):(h + 1) * (HD + 1)],
                    e[:, s5],
                    start=(jb == 0), stop=(jb == NT - 1),
                    skip_group_check=True,
                )

        # ---- prologue: k0 + q0 for token half 0 (x batches 0,1) ----
        # acc slots borrowed from the (still idle) scores ring so four
        # accumulations pipeline; rot slots borrowed from the pv ring.
        accs = []
        for qi in range(2):
            big = ps_s.tile([128, TH2], F32, tag="sT", name="acc_big")
            aK = big[:, 0:512]
            aQ = big[:, 512:1024]
            accs.append((aK, aQ))
            sl = slice(qi * 512, (qi + 1) * 512)
            for k in range(NK):
                last = not with_qkv_bias and k == NK - 1
                nc.tensor.matmul(aK, wqk_sb(k)[:, 2 * 128:3 * 128],
                                 xt(k)[:, sl], start=(k == 0), stop=last,
                                 skip_group_check=True)
                nc.tensor.matmul(aQ, wqk_sb(k)[:, 0:128],
                                 xt(k)[:, sl], start=(k == 0), stop=last,
                                 skip_group_check=True)
            if with_qkv_bias:
                nc.tensor.matmul(aK, t_bqk[:, 2 * 128:3 * 128],
                                 t_ones[:, 0:512], start=False, stop=True,
                                 skip_group_check=True)
                nc.tensor.matmul(aQ, t_bqk[:, 0:128],
                                 t_ones[:, 0:512], start=False, stop=True,
                                 skip_group_check=True)
        # ropes after BOTH batches' accs: the PE chews batch-1 matmuls
        # while the DVE runs batch-0's rope chains instead of idling
        # in-order at a rot matmul.
        for qi in range(2):
            aK, aQ = accs[qi]
            emit_rope(aK, qi, 0, True, rot_ring="pv")
            emit_rope(aQ, qi, 0, False, rot_ring="pv")
        emit_v(0)

        # pending interleave units for pair0's attention stream
        pend = []
        pend.append(lambda: emit_proj_quarter(2, 2, 0, True))   # k0 q2 (jb>=8)
        pend.append(lambda: emit_proj_quarter(2, 3, 0, True))   # k0 q3 (jb>=12)
        pend.append(lambda: emit_proj_quarter(0, 2, 0, False))  # q0 ih1
        pend.append(lambda: emit_proj_quarter(0, 3, 0, False))
        for qi in range(4):
            pend.append(lambda qi=qi: emit_proj_quarter(3, qi, 1, True))   # k1
        for qi in range(4):
            pend.append(lambda qi=qi: emit_proj_quarter(1, qi, 1, False))  # q1

        def drain(n=1):
            for _ in range(n):
                if pend:
                    pend.pop(0)()

        # deferred norm-fins: the ind-matmuls wait ~3us on the reciprocal
        # DMA round-trip; firing them 2 steps into the NEXT quarter keeps
        # them out of the PE queue's critical path.
        fin_box = [None]

        def maybe_fin():
            if fin_box[0] is not None:
                emit_norm_fin(*fin_box[0])
                fin_box[0] = None

        def run_quarter(pair, ih, hh, extra):
            if ih == 0:
                at_t[2 * pair + hh] = p_at.tile([HD + 1, T], F32, tag="aT",
                                                name="at_t")
            at = at_t[2 * pair + hh]
            pvA = ps_pv.tile([HD + 1, 512], F32, tag="pvA", name="pvA_t")
            pvB = ps_pv.tile([HD + 1, 512], F32, tag="pvB", name="pvB_t")
            for jb in range(NT):
                s_ps, e = emit_att_step(pair, ih, hh, jb)
                if jb == 4:
                    maybe_fin()
                extra(jb)
                emit_pv(pair, hh, jb, pvA, pvB, e)
            h0 = ih * TH2
            nc.vector.tensor_copy(at[:, h0:h0 + 512], pvA[:])
            nc.vector.tensor_copy(at[:, h0 + 512:h0 + TH2], pvB[:])
            emit_norm_head(pair, ih, hh)

        # ---- pair 0 attention ----
        def p0_extra(ih, hh):
            def f(jb):
                if ih == 0 and hh == 0:
                    if jb < NT - 1:
                        emit_v(jb + 1)
                    if jb in (1, 5, 9, 13):
                        drain(1)  # k0 q2/q3 ahead of jb 8/12, then q0 ih1
                elif jb % 4 == 0:
                    drain(1)  # k1/q1 quarters, evenly spread
            return f

        for ih in range(2):
            for hh in range(2):
                run_quarter(0, ih, hh, p0_extra(ih, hh))
            fin_box[0] = (0, ih)

        # ---- pair 1 attention ----
        OUTPROJ_SCHED = {(0, 6): 0, (0, 10): 1, (0, 14): 2,
                         (1, 0): 3, (1, 4): 4, (1, 8): 5, (1, 12): 6,
                         (1, 14): 7}

        def p1_extra(ih, hh):
            def f(jb):
                if ih == 0 and jb % 8 == 4:
                    drain(1)  # any leftover proj units
                if ih == 1 and (hh, jb) in OUTPROJ_SCHED:
                    emit_outproj_tile(OUTPROJ_SCHED[(hh, jb)])
            return f

        for ih in range(2):
            for hh in range(2):
                run_quarter(1, ih, hh, p1_extra(ih, hh))
            fin_box[0] = (1, ih)

        # ---- tail: last norm + remaining outproj ----
        maybe_fin()
        for t in range(8, NT):
            emit_outproj_tile(t, tail=True)

    _split_excess_waits(nc)
    return nc


_NC_CACHE = {}


def _rope_tables():
    inv_freq = (1.0 / (ROPE_BASE ** (np.arange(0, HD, 2, dtype=np.float32) / HD))
                ).astype(np.float32)
    t = np.arange(T, dtype=np.float32)
    freqs = np.einsum("t,f->tf", t, inv_freq).astype(np.float32)  # (T, HD/2)
    emb = np.concatenate([freqs, freqs], axis=-1)                  # (T, HD)
    cosT = np.ascontiguousarray(np.cos(emb).astype(np.float32).T)  # (HD, T)
    sinT = np.ascontiguousarray(np.sin(emb).astype(np.float32).T)
    return cosT, sinT


def _rot_matrix():
    r = np.zeros((128, 128), dtype=np.float32)
    for p0 in (0, 64):
        for d in range(32):
            r[p0 + d, p0 + 32 + d] = -1.0
            r[p0 + 32 + d, p0 + d] = 1.0
    return np.ascontiguousarray(r.T)


def kernel(x, W_qkv, b_qkv, W_out, b_out, padding_mask):
    global _NC_CACHE, LAST_RESULTS
    x = np.asarray(x, dtype=np.float32)
    W_qkv = np.asarray(W_qkv, dtype=np.float32)
    b_qkv = np.asarray(b_qkv, dtype=np.float32)
    W_out = np.asarray(W_out, dtype=np.float32)
    b_out = np.asarray(b_out, dtype=np.float32)
    padding_mask = np.asarray(padding_mask)

    with_qkv_bias = bool(np.any(b_qkv[:2 * D]))
    with_v_bias = bool(np.any(b_qkv[2 * D:]))
    key = (with_qkv_bias, with_v_bias)
    if key not in _NC_CACHE:
        _NC_CACHE[key] = _build_bass(with_qkv_bias, with_v_bias)
    nc = _NC_CACHE[key]

    cos2, sin2 = _rope_tables()
    rT = _rot_matrix().astype(BFNP)

    ind = np.zeros((2, 128), dtype=np.float32)
    for f in range(128):
        ind[f // 64, f] = 1.0

    ones = np.ones((1, 512), dtype=BFNP)

    in_maps = []
    for c in range(N_CORES):
        b = c // 4
        g = c % 4
        q0 = g * HL * HD
        wq = W_qkv[:, q0:q0 + HL * HD]
        wk = W_qkv[:, D + q0:D + q0 + HL * HD]
        wv_flat = W_qkv[:, 2 * D + q0:2 * D + q0 + HL * HD]
        # interleave v columns with a zero (ones-slot) column per head
        wv_aug = np.zeros((D, HL * (HD + 1)), dtype=np.float32)
        bv_aug = np.zeros((1, HL * (HD + 1)), dtype=np.float32)
        for h in range(HL):
            wv_aug[:, h * (HD + 1):h * (HD + 1) + HD] = wv_flat[:, h * HD:(h + 1) * HD]
            bv_aug[0, h * (HD + 1):h * (HD + 1) + HD] = \
                b_qkv[2 * D + q0 + h * HD:2 * D + q0 + (h + 1) * HD]
            bv_aug[0, h * (HD + 1) + HD] = 1.0
        bqk = np.concatenate(
            [b_qkv[q0:q0 + HL * HD], b_qkv[D + q0:D + q0 + HL * HD]]
        ).reshape(1, -1).astype(np.float32)
        amask = np.where(padding_mask[b], np.float32(-1e30), np.float32(0.0))
        amask = np.ascontiguousarray(amask.reshape(T // 128, 128).T.astype(np.float32))
        in_maps.append({
            "xT": np.ascontiguousarray(x[b].T).astype(BFNP),
            "wqk": np.ascontiguousarray(
                np.concatenate([wq, wk], axis=1)).astype(BFNP),
            "wv": wv_aug.astype(BFNP),
            "bqk": bqk.astype(BFNP),
            "bv": bv_aug.astype(BFNP),
            "ones": ones,
            "cos2": cos2,
            "sin2": sin2,
            "rT": rT,
            "ind": ind,
            "amask": amask,
            "wo": np.ascontiguousarray(W_out[q0:q0 + HL * HD, :]).astype(BFNP),
        })

    res = bass_utils.run_bass_kernel_spmd(
        nc, in_maps, core_ids=list(range(N_CORES)), trace=TRACE,
    )
    LAST_RESULTS = res

    out = np.zeros((B, T, D), dtype=np.float32)
    for c in range(N_CORES):
        out[c // 4] += res.results[c]["out_part"].astype(np.float32)
    out += b_out.astype(np.float32)
    return out.astype(np.float32)


# revision 39
# speedup vs baseline: 1.0364x; 1.0236x over previous
"""Multi-head self-attention with RoPE on 8 Trainium2 NeuronCores.

Full inputs in, full output out. Sharding: batch (2) x head-groups (4 heads
per core). Each core computes qkv projections for its heads, RoPE, full
softmax(QK^T)V, and a combined (both head-pairs) partial output projection;
host sums the 4 partials per batch element and adds b_out.

All matmul operands are bf16 (fp32 PSUM accumulation); the emission order
interleaves the v projection and pair-1 q/k projections into pair-0's
ACT-bound attention stream so the PE stays busy.

Problem shape: B=2, T=2048, D=1024, H=16, HD=64 (hardcoded).
"""

import numpy as np
from contextlib import ExitStack

import ml_dtypes
import concourse.bass as bass
import concourse.mybir as mybir
import concourse.tile as tile
from concourse import bass_utils

B, T, D, H = 2, 2048, 1024, 16
HD = 64          # head dim
HL = 4           # heads per core
N_CORES = 8
ROPE_BASE = 10000.0

F32 = mybir.dt.float32
F32R = mybir.dt.float32r
BF16 = mybir.dt.bfloat16
BFNP = ml_dtypes.bfloat16

Exp = mybir.ActivationFunctionType.Exp

NT = T // 128     # 16 token tiles
NK = D // 128     # 8 contraction chunks
TH2 = 1024        # query-half width
SC = HD ** -0.5

# results of the last run (for test harness introspection)
LAST_RESULTS = None
TRACE = False


def _split_excess_waits(nc, cap=1):
    """walrus in this env rejects >1 sync-wait per instruction; split extras
    onto single-wait NoOps on the same engine queue."""
    n = 0
    for f in nc.m.functions:
        for bb in f.blocks:
            insts = bb.instructions
            if not any(
                i.sync_info is not None and len(i.sync_info.on_wait) > cap
                for i in insts
            ):
                continue
            out = []
            for inst in insts:
                si = inst.sync_info
                waits = list(si.on_wait) if si is not None else []
                if len(waits) > cap:
                    extra, keep = waits[:-cap], waits[-cap:]
                    for k, w in enumerate(extra):
                        nop = mybir.InstNoOp(
                            name=f"{inst.name}-ws{k}",
                            engine=inst.engine,
                            sync_info=mybir.SyncInfo(on_wait=[w], on_update=[]),
                            bass_nofuse=True,
                        )
                        nc.register_instruction(nop)
                        out.append(nop)
                        n += 1
                    inst.sync_info = mybir.SyncInfo(
                        on_wait=keep, on_update=list(si.on_update)
                    )
                out.append(inst)
            bb.instructions = out
    return n


def _build_bass(with_qkv_bias, with_v_bias):
    nc = bass.Bass("TRN2", target_bir_lowering=False, debug=False, num_devices=1)

    # ---- DRAM I/O ----
    d_xT = nc.dram_tensor("xT", [D, T], BF16, kind="ExternalInput").ap()
    d_wqk = nc.dram_tensor("wqk", [D, 4 * 128], BF16, kind="ExternalInput").ap()
    d_wv = nc.dram_tensor("wv", [D, HL * (HD + 1)], BF16, kind="ExternalInput").ap()
    d_bqk = nc.dram_tensor("bqk", [1, 4 * 128], BF16, kind="ExternalInput").ap()
    d_bv = nc.dram_tensor("bv", [1, HL * (HD + 1)], BF16, kind="ExternalInput").ap()
    d_ones = nc.dram_tensor("ones", [1, 512], BF16, kind="ExternalInput").ap()
    d_cos = nc.dram_tensor("cos2", [HD, T], F32, kind="ExternalInput").ap()
    d_sin = nc.dram_tensor("sin2", [HD, T], F32, kind="ExternalInput").ap()
    d_rT = nc.dram_tensor("rT", [128, 128], BF16, kind="ExternalInput").ap()
    d_ind = nc.dram_tensor("ind", [2, 128], F32R, kind="ExternalInput").ap()
    d_amask = nc.dram_tensor("amask", [128, NT], F32, kind="ExternalInput").ap()
    d_wo = nc.dram_tensor("wo", [2 * 128, D], BF16, kind="ExternalInput").ap()
    d_out = nc.dram_tensor("out_part", [T, D], BF16, kind="ExternalOutput").ap()

    with tile.TileContext(nc) as tc, ExitStack() as ctx:
        pool = lambda name, bufs: ctx.enter_context(tc.tile_pool(name=name, bufs=bufs))
        psum = lambda name, bufs: ctx.enter_context(
            tc.tile_pool(name=name, bufs=bufs, space="PSUM")
        )

        p_const = pool("const", 1)
        p_xt = pool("xt", NK)
        p_w = pool("w", NK)
        p_wv = pool("wv", NK)
        p_cs = pool("cs", 1)
        p_tmp = pool("tmp", 2)
        p_qk = pool("qk", 2)
        p_v = pool("v", NT)
        p_e = pool("e", 4)
        p_at = pool("at", 4)
        p_fin = pool("fin", 2)

        ps_s = psum("ps_s", 2)      # [128,1024] f32 -> 4 banks
        ps_pv = psum("ps_pv", 1)    # [65,1024] f32 -> 2 banks
        ps_aux = psum("ps_aux", 2)  # [128,512] f32 -> 2 banks

        # ---- input loads ----
        # x arrives in column-batches of 512 tokens via big rearranged
        # descriptors: batch qi unlocks the full contraction for token
        # quarter qi across every projection, so the PE starts ~8us in.
        # wqk rides the sync ring first; tables on the scalar ring.
        xt_all = p_xt.tile([128, NK * T], BF16, tag="xt", bufs=1, name="xt_all")
        wqk_all = p_w.tile([128, NK * 512], BF16, tag="wqk", bufs=1,
                           name="wqk_all")
        xt3 = xt_all[:].rearrange("p (c w) -> p c w", c=NK)
        xsrc = d_xT[:].rearrange("(c p) w -> p c w", p=128)
        # quarter 0 gates the whole prologue: cheap 2-D per-chunk
        # descriptors, wqk chunk interleaved with its x chunk so chunk-k
        # matmuls start as soon as pair k lands.
        for k in range(NK):
            nc.sync.dma_start(wqk_all[:, k * 512:(k + 1) * 512],
                              d_wqk[k * 128:(k + 1) * 128, :])
            nc.sync.dma_start(xt3[:, k, 0:512], xsrc[:, k, 0:512])
        for qi in range(1, 4):
            ws = slice(qi * 512, (qi + 1) * 512)
            nc.sync.dma_start(xt3[:, :, ws], xsrc[:, :, ws])

        def xt(k):
            return xt_all[:, k * T:(k + 1) * T]

        def wqk_sb(k):
            return wqk_all[:, k * 512:(k + 1) * 512]

        t_rT = p_const.tile([128, 128], BF16, tag="rT")
        nc.scalar.dma_start(t_rT[:], d_rT[:])
        t_cos = p_cs.tile([128, T], F32, tag="cos")
        t_sin = p_cs.tile([128, T], F32, tag="sin")
        nc.scalar.dma_start(t_sin[0:HD, :], d_sin[:])
        nc.scalar.dma_start(t_cos[0:HD, :], d_cos[:])
        nc.scalar.dma_start(t_sin[HD:128, :], t_sin[0:HD, :])
        nc.scalar.dma_start(t_cos[HD:128, :], t_cos[0:HD, :])
        t_amask = p_const.tile([128, NT], F32, tag="amask")
        nc.scalar.dma_start(t_amask[:], d_amask[:])
        t_indA = p_const.tile([1, 128], F32R, tag="indA")
        nc.scalar.dma_start(t_indA[:], d_ind[0:1, :])
        t_indB = p_const.tile([1, 128], F32R, tag="indB")
        nc.scalar.dma_start(t_indB[:], d_ind[1:2, :])
        t_ones = p_const.tile([1, 512], BF16, tag="ones")
        nc.scalar.dma_start(t_ones[:], d_ones[:])
        t_bqk = p_const.tile([1, 4 * 128], BF16, tag="bqk")
        nc.scalar.dma_start(t_bqk[:], d_bqk[:])
        t_bv = p_const.tile([1, HL * (HD + 1)], BF16, tag="bv")
        nc.scalar.dma_start(t_bv[:], d_bv[:])
        wv_all = p_wv.tile([128, NK * 260], BF16, tag="wv", bufs=1,
                           name="wv_all")
        nc.scalar.dma_start(
            wv_all[:].rearrange("p (c w) -> p c w", c=NK),
            d_wv[:].rearrange("(c p) w -> p c w", p=128),
        )

        def wv_sb(k):
            return wv_all[:, k * 260:(k + 1) * 260]

        # out-proj weights: not needed until late; sync ring after x
        wo_sb = []
        for c2 in range(2):
            wt = p_fin.tile([128, D], BF16, tag="wo", name="wo_t")
            nc.sync.dma_start(wt[:], d_wo[c2 * 128:(c2 + 1) * 128, :])
            wo_sb.append(wt)

        # ---- persistent q/k tiles; zero-pad k halves once ----
        qc, kA, kB = [], [], []
        for pair in range(2):
            tq = p_qk.tile([128, T], BF16, tag="qc", name="qc_t")
            ta = p_qk.tile([128, T], BF16, tag="kA", name="kA_t")
            tb = p_qk.tile([128, T], BF16, tag="kB", name="kB_t")
            nc.gpsimd.memset(ta[HD:128, :], 0.0)
            nc.gpsimd.memset(tb[0:HD, :], 0.0)
            qc.append(tq)
            kA.append(ta)
            kB.append(tb)

        v_sb = [None] * NT
        at_t = [None] * 4
        an_t = [None] * 4  # (pair, ih) -> 2*pair + ih

        # ---- emission helpers ----
        def emit_proj_mms(acc, c2, sl):
            for k in range(NK):
                nc.tensor.matmul(
                    acc,
                    wqk_sb(k)[:, c2 * 128:(c2 + 1) * 128],
                    xt(k)[:, sl],
                    start=(k == 0),
                    stop=(not with_qkv_bias and k == NK - 1),
                    skip_group_check=True,
                )
            if with_qkv_bias:
                nc.tensor.matmul(
                    acc,
                    t_bqk[:, c2 * 128:(c2 + 1) * 128],
                    t_ones[:, 0:512],
                    start=False,
                    stop=True,
                    skip_group_check=True,
                )

        def emit_rope(acc, qi, pair, is_k, rot_ring="aux"):
            """RoPE: roped = raw*cos + R @ (raw*sin); store q/k bf16."""
            sl = slice(qi * 512, (qi + 1) * 512)
            u = p_tmp.tile([128, 512], BF16, tag="u", name="u_t")
            nc.vector.tensor_mul(u[:], acc, t_sin[:, sl])
            if rot_ring == "pv":
                rot = ps_pv.tile([128, 512], F32, tag="pvA", name="rot")
            else:
                rot = ps_aux.tile([128, 512], F32, tag="aux", name="rot")
            nc.tensor.matmul(rot[:], t_rT[:], u[:], start=True, stop=True,
                             skip_group_check=True)
            c_sb = p_tmp.tile([128, 512], F32, tag="c", name="c_t")
            nc.vector.tensor_mul(c_sb[:], acc, t_cos[:, sl])
            if not is_k:
                nc.vector.tensor_add(qc[pair][:, sl], c_sb[:], rot[:])
            else:
                nc.vector.tensor_add(kA[pair][0:HD, sl], c_sb[0:HD, :],
                                     rot[0:HD, :])
                nc.vector.tensor_add(kB[pair][HD:128, sl], c_sb[HD:128, :],
                                     rot[HD:128, :])

        def emit_proj_quarter(c2, qi, pair, is_k):
            """interleaved-unit variant: acc+rot from the aux ring."""
            acc = ps_aux.tile([128, 512], F32, tag="aux", name="acc")
            emit_proj_mms(acc[:], c2, slice(qi * 512, (qi + 1) * 512))
            emit_rope(acc[:], qi, pair, is_k, rot_ring="aux")

        def emit_v_acc(j):
            acc = ps_aux.tile([128, 512], F32, tag="aux", name="vacc")
            av = acc[:, 0:HL * (HD + 1)]
            for k in range(NK):
                nc.tensor.matmul(
                    av,
                    xt(k)[:, j * 128:(j + 1) * 128],
                    wv_sb(k)[:],
                    start=(k == 0),
                    stop=(not with_v_bias and k == NK - 1),
                    skip_group_check=True,
                )
            if with_v_bias:
                nc.tensor.matmul(av, t_ones[:, 0:128], t_bv[:],
                                 start=False, stop=True, skip_group_check=True)
            return acc

        def emit_v_fin(j, acc):
            av = acc[:, 0:HL * (HD + 1)]
            vt = p_v.tile([128, HL * (HD + 1)], BF16, tag="v", name="v_t")
            nc.vector.tensor_copy(vt[:], av)
            if not with_v_bias:
                ones_cols = vt[:].rearrange("p (h c) -> p h c", h=HL)[:, :, HD:HD + 1]
                nc.gpsimd.memset(ones_cols, 1.0)
            v_sb[j] = vt

        def emit_v(j):
            emit_v_fin(j, emit_v_acc(j))

        norm_state = {}

        def emit_norm_head(pair, ih, hh):
            """per-head normalization prep right after head hh's attention
            half: reciprocal of the denominator row straight from PSUM, and
            (head B only) a partition-shift gather of its attention output."""
            at_ = at_t[2 * pair + hh]
            # reciprocal needs a partition-spread layout: gather the
            # denominator row, recip, scatter back flat — per 512-token
            # half, so each path starts after its own at-copy half.
            sums = p_fin.tile([128, 8], F32, tag="sums", bufs=4, name="sums_t")
            rec = p_fin.tile([128, 8], F32, tag="rec", bufs=4, name="rec_t")
            rrow = p_fin.tile([1, TH2], F32R, tag="rrow", bufs=4, name="rrow_t")
            ar = None
            if hh == 1:
                ar = p_fin.tile([128, TH2], F32, tag="ar", name="ar_t")
            for n5 in range(2):
                cs = slice(n5 * 4, (n5 + 1) * 4)
                h5 = slice(n5 * 512, (n5 + 1) * 512)
                ssl = slice(ih * TH2 + n5 * 512, ih * TH2 + (n5 + 1) * 512)
                nc.sync.dma_start(
                    sums[:, cs],
                    at_[HD:HD + 1, ssl].rearrange("o (p c) -> o p c", p=128))
                nc.vector.reciprocal(rec[:, cs], sums[:, cs])
                nc.sync.dma_start(
                    rrow[:, h5].rearrange("o (p c) -> o p c", p=128),
                    rec[:, cs].bitcast(F32R))
                if hh == 1:
                    nc.sync.dma_start(ar[HD:128, h5], at_[0:HD, ssl])
            if hh == 0:
                norm_state[(pair, ih)] = [rrow, None]
            else:
                norm_state[(pair, ih)][1] = rrow
                norm_state[(pair, ih)].append(ar)

        def emit_norm_fin(pair, ih):
            rrowA, rrowB, arB = norm_state.pop((pair, ih))
            at0 = at_t[2 * pair]
            hsl = slice(ih * TH2, (ih + 1) * TH2)
            an = p_fin.tile([128, TH2], BF16, tag="an", bufs=4, name="an_t")
            for n5 in range(2):
                s5 = slice(n5 * 512, (n5 + 1) * 512)
                g5 = slice(ih * TH2 + n5 * 512, ih * TH2 + (n5 + 1) * 512)
                pb = ps_aux.tile([128, 512], F32, tag="aux", name="pb")
                nc.tensor.matmul(pb[:], t_indA[:], rrowA[:, s5],
                                 start=True, stop=False, skip_group_check=True)
                nc.tensor.matmul(pb[:], t_indB[:], rrowB[:, s5],
                                 start=False, stop=True, skip_group_check=True)
                nc.vector.tensor_mul(an[0:HD, s5], pb[0:HD, :], at0[0:HD, g5])
                nc.vector.tensor_mul(an[HD:128, s5], pb[HD:128, :],
                                     arB[HD:128, s5])
            an_t[2 * pair + ih] = an

        def emit_outproj_tile(t, tail=False):
            """output projection for token tile t, both pairs accumulated."""
            ih = t // 8
            off = (t % 8) * 128
            an0, an1 = an_t[0 + ih], an_t[2 + ih]
            osb = p_fin.tile([128, D], BF16, tag="osb", bufs=4, name="osb_t")
            for n5 in range(2):
                s5 = slice(n5 * 512, (n5 + 1) * 512)
                pp = ps_aux.tile([128, 512], F32, tag="aux", name="pp")
                nc.tensor.matmul(pp[:], an0[:, off:off + 128], wo_sb[0][:, s5],
                                 start=True, stop=False, skip_group_check=True)
                nc.tensor.matmul(pp[:], an1[:, off:off + 128], wo_sb[1][:, s5],
                                 start=False, stop=True, skip_group_check=True)
                if tail:
                    # ACT is idle after the last exp; keep DVE free too
                    nc.scalar.copy(osb[:, s5], pp[:])
                else:
                    nc.vector.tensor_copy(osb[:, s5], pp[:])
            nc.sync.dma_start(d_out[t * 128:(t + 1) * 128, :], osb[:])

        def emit_att_step(pair, ih, hh, jb):
            kp = (kA, kB)[hh][pair]
            s_ps = ps_s.tile([128, TH2], F32, tag="sT", name="s_ps")
            for n5 in range(2):
                s5 = slice(n5 * 512, (n5 + 1) * 512)
                g5 = slice(ih * TH2 + n5 * 512, ih * TH2 + (n5 + 1) * 512)
                nc.tensor.matmul(
                    s_ps[:, s5], kp[:, jb * 128:(jb + 1) * 128], qc[pair][:, g5],
                    start=True, stop=True, skip_group_check=True,
                )
            e = p_e.tile([128, TH2], BF16, tag="e", name="e_t")
            nc.scalar.activation(e[:], s_ps[:], Exp,
                                 bias=t_amask[:, jb:jb + 1], scale=SC)
            return s_ps, e

        def emit_pv(pair, hh, jb, pvA, pvB, e):
            h = 2 * pair + hh
            for n5, pvh in ((0, pvA), (1, pvB)):
                s5 = slice(n5 * 512, (n5 + 1) * 512)
                nc.tensor.matmul(
                    pvh[:],
                    v_sb[jb][:, h * (HD + 1):(h + 1) * (HD + 1)],
                    e[:, s5],
                    start=(jb == 0), stop=(jb == NT - 1),
                    skip_group_check=True,
                )

        # ---- prologue: k0 + q0 for token half 0 (x batches 0,1) ----
        # acc slots borrowed from the (still idle) scores ring so four
        # accumulations pipeline; rot slots borrowed from the pv ring.
        accs = []
        for qi in range(2):
            big = ps_s.tile([128, TH2], F32, tag="sT", name="acc_big")
            aK = big[:, 0:512]
            aQ = big[:, 512:1024]
            accs.append((aK, aQ))
            sl = slice(qi * 512, (qi + 1) * 512)
            for k in range(NK):
                last = not with_qkv_bias and k == NK - 1
                nc.tensor.matmul(aK, wqk_sb(k)[:, 2 * 128:3 * 128],
                                 xt(k)[:, sl], start=(k == 0), stop=last,
                                 skip_group_check=True)
                nc.tensor.matmul(aQ, wqk_sb(k)[:, 0:128],
                                 xt(k)[:, sl], start=(k == 0), stop=last,
                                 skip_group_check=True)
            if with_qkv_bias:
                nc.tensor.matmul(aK, t_bqk[:, 2 * 128:3 * 128],
                                 t_ones[:, 0:512], start=False, stop=True,
                                 skip_group_check=True)
                nc.tensor.matmul(aQ, t_bqk[:, 0:128],
                                 t_ones[:, 0:512], start=False, stop=True,
                                 skip_group_check=True)
        # ropes after BOTH batches' accs (PE chews batch-1 matmuls while
        # DVE runs batch-0's chains), ordered by first use: jb0 needs both
        # q0 quarters + k0-quarter0; k0-quarter1 isn't read until jb4.
        emit_rope(accs[0][0], 0, 0, True, rot_ring="pv")
        emit_rope(accs[0][1], 0, 0, False, rot_ring="pv")
        emit_rope(accs[1][1], 1, 0, False, rot_ring="pv")
        emit_v(0)
        emit_rope(accs[1][0], 1, 0, True, rot_ring="pv")

        # pending interleave units for pair0's attention stream
        pend = []
        pend.append(lambda: emit_proj_quarter(2, 2, 0, True))   # k0 q2 (jb>=8)
        pend.append(lambda: emit_proj_quarter(2, 3, 0, True))   # k0 q3 (jb>=12)
        pend.append(lambda: emit_proj_quarter(0, 2, 0, False))  # q0 ih1
        pend.append(lambda: emit_proj_quarter(0, 3, 0, False))
        for qi in range(4):
            pend.append(lambda qi=qi: emit_proj_quarter(3, qi, 1, True))   # k1
        for qi in range(4):
            pend.append(lambda qi=qi: emit_proj_quarter(1, qi, 1, False))  # q1

        def drain(n=1):
            for _ in range(n):
                if pend:
                    pend.pop(0)()

        # deferred norm-fins: the ind-matmuls wait ~3us on the reciprocal
        # DMA round-trip; firing them 2 steps into the NEXT quarter keeps
        # them out of the PE queue's critical path.
        fin_box = [None]

        def maybe_fin():
            if fin_box[0] is not None:
                emit_norm_fin(*fin_box[0])
                fin_box[0] = None

        def run_quarter(pair, ih, hh, extra):
            if ih == 0:
                at_t[2 * pair + hh] = p_at.tile([HD + 1, T], F32, tag="aT",
                                                name="at_t")
            at = at_t[2 * pair + hh]
            pvA = ps_pv.tile([HD + 1, 512], F32, tag="pvA", name="pvA_t")
            pvB = ps_pv.tile([HD + 1, 512], F32, tag="pvB", name="pvB_t")
            for jb in range(NT):
                s_ps, e = emit_att_step(pair, ih, hh, jb)
                if jb == 4:
                    maybe_fin()
                extra(jb)
                emit_pv(pair, hh, jb, pvA, pvB, e)
            h0 = ih * TH2
            nc.vector.tensor_copy(at[:, h0:h0 + 512], pvA[:])
            nc.vector.tensor_copy(at[:, h0 + 512:h0 + TH2], pvB[:])
            emit_norm_head(pair, ih, hh)

        # ---- pair 0 attention ----
        def p0_extra(ih, hh):
            def f(jb):
                if ih == 0 and hh == 0:
                    if jb < NT - 1:
                        emit_v(jb + 1)
                    if jb in (1, 5, 9, 13):
                        drain(1)  # k0 q2/q3 ahead of jb 8/12, then q0 ih1
                elif jb % 4 == 0:
                    drain(1)  # k1/q1 quarters, evenly spread
            return f

        for ih in range(2):
            for hh in range(2):
                run_quarter(0, ih, hh, p0_extra(ih, hh))
            fin_box[0] = (0, ih)

        # ---- pair 1 attention ----
        OUTPROJ_SCHED = {(0, 6): 0, (0, 10): 1, (0, 14): 2,
                         (1, 0): 3, (1, 4): 4, (1, 8): 5, (1, 12): 6,
                         (1, 14): 7}

        def p1_extra(ih, hh):
            def f(jb):
                if ih == 0 and jb % 8 == 4:
                    drain(1)  # any leftover proj units
                if ih == 1 and (hh, jb) in OUTPROJ_SCHED:
                    emit_outproj_tile(OUTPROJ_SCHED[(hh, jb)])
            return f

        for ih in range(2):
            for hh in range(2):
                run_quarter(1, ih, hh, p1_extra(ih, hh))
            fin_box[0] = (1, ih)

        # ---- tail: last norm + remaining outproj ----
        maybe_fin()
        for t in range(8, NT):
            emit_outproj_tile(t, tail=True)

    _split_excess_waits(nc)
    return nc


_NC_CACHE = {}


def _rope_tables():
    inv_freq = (1.0 / (ROPE_BASE ** (np.arange(0, HD, 2, dtype=np.float32) / HD))
                ).astype(np.float32)
    t = np.arange(T, dtype=np.float32)
    freqs = np.einsum("t,f->tf", t, inv_freq).astype(np.float32)  # (T, HD/2)
    emb = np.concatenate([freqs, freqs], axis=-1)                  # (T, HD)
    cosT = np.ascontiguousarray(np.cos(emb).astype(np.float32).T)  # (HD, T)
    sinT = np.ascontiguousarray(np.sin(emb).astype(np.float32).T)
    return cosT, sinT


def _rot_matrix():
    r = np.zeros((128, 128), dtype=np.float32)
    for p0 in (0, 64):
        for d in range(32):
            r[p0 + d, p0 + 32 + d] = -1.0
            r[p0 + 32 + d, p0 + d] = 1.0
    return np.ascontiguousarray(r.T)


def kernel(x, W_qkv, b_qkv, W_out, b_out, padding_mask):
    global _NC_CACHE, LAST_RESULTS
    x = np.asarray(x, dtype=np.float32)
    W_qkv = np.asarray(W_qkv, dtype=np.float32)
    b_qkv = np.asarray(b_qkv, dtype=np.float32)
    W_out = np.asarray(W_out, dtype=np.float32)
    b_out = np.asarray(b_out, dtype=np.float32)
    padding_mask = np.asarray(padding_mask)

    with_qkv_bias = bool(np.any(b_qkv[:2 * D]))
    with_v_bias = bool(np.any(b_qkv[2 * D:]))
    key = (with_qkv_bias, with_v_bias)
    if key not in _NC_CACHE:
        _NC_CACHE[key] = _build_bass(with_qkv_bias, with_v_bias)
    nc = _NC_CACHE[key]

    cos2, sin2 = _rope_tables()
    rT = _rot_matrix().astype(BFNP)

    ind = np.zeros((2, 128), dtype=np.float32)
    for f in range(128):
        ind[f // 64, f] = 1.0

    ones = np.ones((1, 512), dtype=BFNP)

    in_maps = []
    for c in range(N_CORES):
        b = c // 4
        g = c % 4
        q0 = g * HL * HD
        wq = W_qkv[:, q0:q0 + HL * HD]
        wk = W_qkv[:, D + q0:D + q0 + HL * HD]
        wv_flat = W_qkv[:, 2 * D + q0:2 * D + q0 + HL * HD]
        # interleave v columns with a zero (ones-slot) column per head
        wv_aug = np.zeros((D, HL * (HD + 1)), dtype=np.float32)
        bv_aug = np.zeros((1, HL * (HD + 1)), dtype=np.float32)
        for h in range(HL):
            wv_aug[:, h * (HD + 1):h * (HD + 1) + HD] = wv_flat[:, h * HD:(h + 1) * HD]
            bv_aug[0, h * (HD + 1):h * (HD + 1) + HD] = \
                b_qkv[2 * D + q0 + h * HD:2 * D + q0 + (h + 1) * HD]
            bv_aug[0, h * (HD + 1) + HD] = 1.0
        bqk = np.concatenate(
            [b_qkv[q0:q0 + HL * HD], b_qkv[D + q0:D + q0 + HL * HD]]
        ).reshape(1, -1).astype(np.float32)
        amask = np.where(padding_mask[b], np.float32(-1e30), np.float32(0.0))
        amask = np.ascontiguousarray(amask.reshape(T // 128, 128).T.astype(np.float32))
        in_maps.append({
            "xT": np.ascontiguousarray(x[b].T).astype(BFNP),
            "wqk": np.ascontiguousarray(
                np.concatenate([wq, wk], axis=1)).astype(BFNP),
            "wv": wv_aug.astype(BFNP),
            "bqk": bqk.astype(BFNP),
            "bv": bv_aug.astype(BFNP),
            "ones": ones,
            "cos2": cos2,
            "sin2": sin2,
            "rT": rT,
            "ind": ind,
            "amask": amask,
            "wo": np.ascontiguousarray(W_out[q0:q0 + HL * HD, :]).astype(BFNP),
        })

    res = bass_utils.run_bass_kernel_spmd(
        nc, in_maps, core_ids=list(range(N_CORES)), trace=TRACE,
    )
    LAST_RESULTS = res

    out = np.zeros((B, T, D), dtype=np.float32)
    for c in range(N_CORES):
        out[c // 4] += res.results[c]["out_part"].astype(np.float32)
    out += b_out.astype(np.float32)
    return out.astype(np.float32)
